# revision 11
# baseline (speedup 1.0000x reference)
"""Trainium2 Bass kernel for per-sample dynamic 3x3 conv (periodic padding).

y[b,o,h,w] = sum_{c,i,j} x[b,c,(h+i-1)%H,(w+j-1)%W] * wgt[b, c*9+i*3+j, o] + bias[b,o]

Shapes: x [16,64,128,128] f32, wgt [16,576,64] f32, bias [16,64] f32.
Sharding: data-parallel over batch, 2 samples per core on 8 cores.

Compute scheme: 64x64 PE-array tiling. Every matmul is K=64 (C), M=64 (O);
the four quadrant matmuls run concurrently on the 128x128 array (measured
216ns per 512-column tap block = 1 col/cycle/quadrant at warm clock).
Quadrant (s,g): sample s on array rows 64s:64s+64, col-group g computes
output rows 4g:4g+4 of an 8-row spatial tile; each quadrant owns a private
(partition x bank) PSUM region so the four accumulation chains start/stop
independently. Tiles are processed in pairs with the tap loop outer within
the pair so _dedup_ldweights drops the second tile's redundant InstLdweights
(bf16 weights stay resident; LDWs background-load under the previous tap's
stream).

The wrap padding is materialized HOST-side ([2,64,130,130] bf16 input), so
every (tap, tile, quadrant) is one clean N=512 matmul - no 1-wide column
slivers (+26ns per affected tap block) and no boundary-tile row splits.

Head: image/weights/bias live in RAW sbuf tensors loaded by DMAs issued
BEFORE the TileContext entry barrier (saves the ~0.8us poison-sem dance),
with manual completion semaphores; the PE stream takes one explicit wait
per row segment, placed right before the first matmul of the first tile
that needs it. The critical pieces (weights + rows 0:11) ride the sync
HWDGE ring, which measured ~0.8us first-touch latency vs ~2us for the
scalar ring. Eight warm-up matmuls on a zeroed scratch (into a PSUM bank
that is freed before the pools allocate; safe because the PE queue is
in-order) keep the PE busy through the DMA head so the HAM clock-gate is
at 2.4GHz when the real stream starts. The ACT table preload runs off a
const input pre-context.

Evacuation per sample: the g=s quadrant is partition-aligned with the
output slot -> DVE tensor_scalar_add(+bias); the g=1-s quadrant crosses
partitions -> ACT activation(Identity, +bias). Tile dep tracking is
partition-blind, so DVE/ACT writes to the same st rows serialize; emitting
both DVE ops before both ACT ops makes every cross-engine edge point
DVE->ACT (never ACT->DVE), keeping the drain to one DVE latency. Stores
alternate between the sync and scalar HWDGE rings (the gpsimd SWDGE ring
measured only ~120GB/s and back-pressured the whole pipeline); the last
tile stores as two 4-row halves on both rings to shorten the final drain.
Inputs/weights are cast to bf16 host-side; output is stored bf16 and
upcast on the host.
"""

import numpy as np

KH = KW = 3
B, C, O, H, W = 16, 64, 64, 128, 128
HP, WP = H + 2, W + 2  # host-padded image
N_CORES = 8
BPC = B // N_CORES  # samples per core
TILE_ROWS = 8  # output rows per spatial tile (4 per quadrant col-group)
QROWS = TILE_ROWS // 2  # rows per quadrant -> N = 4*128 = 512
N_TILES = H // TILE_ROWS
G = 2  # spatial tiles per tap-outer pair (LDW reuse span)
N_WARMUP_MM = 8  # HAM warm-up matmuls (~3.4us cold) hidden under the DMA head

TAPS = [(i, j) for i in range(KH) for j in range(KW)]

# (row_lo, row_hi, ring) row segments of the padded image; tile t needs
# padded rows [8t, 8t+10].
SEGS = [
    (0, 11, "sync"),
    (11, 19, "scalar"),
    (19, 35, "sync"),
    (35, 59, "sync"),
    (59, 91, "scalar"),
    (91, HP, "sync"),
]

_CACHE = {}


def _patch_tile_drain():
    """This container's walrus rejects Drain instructions carrying more than
    one sem wait (setupSyncWait: Too many sync wait commands). Re-emit the
    TileContext exit drain's waits as individual wait_ge instructions."""
    import concourse.tile as tile
    from concourse.vector_clock import ScopedClock

    if getattr(tile.TileContext, "_drain_patch_applied", False):
        return

    def _drain_and_barrier(self, tick_clock, wait_clock):
        import concourse.mybir as mybir

        nc = self.nc
        nop = nc.sync.nop(nofuse=True)
        wait_clock.add_sem_waits(nop.ins, ScopedClock({None: tick_clock.global_clock}))
        waits = list(nop.ins.sync_info.on_wait)
        nop.ins.sync_info.on_wait.clear()
        assert self.sems is not None
        by_name = {}
        for h in self.sems.allocated().values():
            by_name[getattr(h, "name", None)] = h
        # Spread the final sem waits round-robin over all engine queues
        # (serial ~115ns/wait on one queue otherwise); the sem-only barrier
        # below restores the all-engines ordering.
        engs = [nc.sync, nc.vector, nc.scalar, nc.tensor, nc.gpsimd]
        for k, w in enumerate(waits):
            h = by_name.get(w.ant_name)
            assert h is not None, f"no sem handle for {w.ant_name}"
            engs[k % len(engs)].wait_ge(h, w.wait_value)
        # per-engine drains except GpSimd's expensive dge_drain, then a
        # sem-only barrier (mirrors BassBlock.no_gpsimd_drain)
        gpsimd_type = nc.gpsimd.engine
        for eng_type, eng in nc.engines.items():
            if eng_type == gpsimd_type:
                continue
            d = mybir.InstDrain(
                name=nc.get_next_instruction_name(),
                ins=[],
                outs=[],
                bass_is_fusable=False,
            )
            d.engine = eng_type
            eng.add_instruction(d)
        nc.all_engine_barrier(sem_only=True)
        popped = nc._tile_sem_poison_stack.pop()
        assert popped is self._sem_poison
        # Skip the exit-time gpsimd dma_reset+sem_clear: the kernel preamble
        # already clears the whole kernel sem range on every execution, and
        # the gpsimd dge drain it implies costs microseconds. Only free the
        # IDs python-side (no further tiles are built after this point).
        sems = list(self.sems.allocated().values())
        sem_nums = [s.num for s in sems]
        nc._state.prepend_free_semaphores(sem_nums)
        for poison_set in nc._tile_sem_poison_stack:
            poison_set.update(sem_nums)

    tile.TileContext._drain_and_barrier = _drain_and_barrier
    tile.TileContext._drain_patch_applied = True


def _split_multi_waits(nc, max_waits=1):
    """Same walrus limitation, general form: any instruction carrying more
    than one sem wait fails setupSyncWait. Hoist excess waits onto dedicated
    single-wait NOPs on the same engine, placed just before the instruction."""
    import concourse.mybir as mybir

    for f in nc.m.functions:
        for blk in f.blocks:
            out = []
            changed = False
            for inst in blk.instructions:
                si = getattr(inst, "sync_info", None)
                waits = list(si.on_wait) if si is not None else []
                if len(waits) > max_waits:
                    changed = True
                    for w in waits[:-max_waits]:
                        out.append(
                            mybir.InstNoOp(
                                name=nc.get_next_instruction_name(),
                                engine=inst.engine,
                                sync_info=mybir.SyncInfo(on_wait=[w], on_update=[]),
                                bass_nofuse=True,
                            )
                        )
                    si.on_wait.clear()
                    for w in waits[-max_waits:]:
                        si.on_wait.append(w)
                out.append(inst)
            if changed:
                blk.instructions = out


def _dedup_ldweights(nc):
    """Drop InstLdweights that reload the exact weights already resident in
    the same PE array quadrant (bf16 weights persist across matmuls; a
    non-self-loading InstMatmult then reuses them - the pattern
    nc.tensor.ldweights documents as supported for 16-bit dtypes). Sync
    carried by a dropped load moves onto the next kept PE instruction."""
    import concourse.mybir as mybir

    def key(inst):
        a = inst.ins[0]
        return (
            a.memref,
            a.offset,
            tuple(tuple(d) for d in a.ap),
            tuple(inst.tile_position or (0, 0)),
            inst.perf_mode,
            inst.is_transpose,
        )

    dropped = 0
    for f in nc.m.functions:
        for blk in f.blocks:
            last = {}
            out = []
            pend_waits = []
            pend_updates = []
            for inst in blk.instructions:
                if getattr(inst, "engine", None) != mybir.EngineType.PE:
                    out.append(inst)
                    continue
                if isinstance(inst, mybir.InstLdweights):
                    pos = tuple(inst.tile_position or (0, 0))
                    k = key(inst)
                    if last.get(pos) == k:
                        si = inst.sync_info
                        if si is not None:
                            pend_waits.extend(si.on_wait)
                            pend_updates.extend(si.on_update)
                        dropped += 1
                        continue
                    last[pos] = k
                elif not isinstance(inst, mybir.InstMatmult):
                    last = {}  # unknown PE inst: conservatively forget
                if pend_waits or pend_updates:
                    if inst.sync_info is None:
                        inst.sync_info = mybir.SyncInfo(on_wait=[], on_update=[])
                    for w in pend_waits:
                        inst.sync_info.on_wait.append(w)
                    for u in pend_updates:
                        inst.sync_info.on_update.append(u)
                    pend_waits, pend_updates = [], []
                out.append(inst)
            assert not pend_waits and not pend_updates
            blk.instructions = out
    return dropped


def _build_module():
    import concourse.bass as bass
    import concourse.mybir as mybir
    import concourse.tile as tile

    _patch_tile_drain()

    f32 = mybir.dt.float32
    bf16 = mybir.dt.bfloat16

    nc = bass.Bass()
    x_d = nc.dram_tensor("input", [BPC, C, HP, WP], bf16, kind="ExternalInput")
    # weights pre-transposed host-side: wts[64*b+c, tap, o]
    w_d = nc.dram_tensor("wts", [128, KH * KW, O], bf16, kind="ExternalInput")
    b_d = nc.dram_tensor("bias", [BPC, O], f32, kind="ExternalInput")
    y_d = nc.dram_tensor("out", [BPC, O, H, W], bf16, kind="ExternalOutput")

    x_bc = x_d.rearrange("b c h w -> (b c) h w")
    y_bo = y_d.rearrange("b o h w -> (b o) h w")

    # --- raw persistent sbuf state, loaded pre-TileContext with manual sems
    raw = nc.alloc_sbuf_tensor("raw", [128, HP, WP], bf16)
    wts = nc.alloc_sbuf_tensor("wtsb", [128, KH * KW, O], bf16)
    bias_sb = nc.alloc_sbuf_tensor("biasb", [128, 1], f32)
    warm = nc.alloc_sbuf_tensor("warm", [128, 512], bf16)
    scratch1 = nc.alloc_sbuf_tensor("scratch1", [128, 1], f32)

    s_wts = nc.alloc_semaphore("s_wts")
    s_bias = nc.alloc_semaphore("s_bias")
    s_warm = nc.alloc_semaphore("s_warm")
    s_seg = [nc.alloc_semaphore(f"s_seg{k}") for k in range(len(SEGS))]

    const0 = nc.const_aps.aps[(f32, 0.0)]

    nc.gpsimd.memset(warm[:, :], 0).then_inc(s_warm, 1)

    # critical loads on the sync HWDGE ring (fast first-touch), the rest
    # interleaved; bias + one segment keep the scalar ring's slow first
    # transfer off the critical path.
    nc.sync.dma_start(out=wts[:, :, :], in_=w_d[:, :, :]).then_inc(s_wts, 16)
    ring = {"sync": nc.sync, "scalar": nc.scalar}
    for k, (lo, hi, eng) in enumerate(SEGS):
        ring[eng].dma_start(out=raw[:, lo:hi, :], in_=x_bc[:, lo:hi, :]).then_inc(
            s_seg[k], 16
        )
        if k == 0:
            nc.scalar.dma_start(
                out=bias_sb[:, :],
                in_=b_d.rearrange("b o -> (b o)").rearrange("(p x) -> p x", x=1),
            ).then_inc(s_bias, 16)
            # ACT table preload off a const input (~1.3us, hidden in head)
            nc.scalar.activation(
                out=scratch1[:, :],
                in_=const0,
                func=mybir.ActivationFunctionType.Identity,
                bias=const0,
            )

    # --- PE warm-up on zeros; the PSUM bank is freed before the pools
    # allocate (in-order PE queue makes the overlap safe).
    with nc.psum_tensor([128, 512]) as wps:
        nc.tensor.wait_ge(s_warm, 1)
        for _ in range(N_WARMUP_MM):
            nc.tensor.matmul(
                wps[:, :], lhsT=warm[:, 0:128], rhs=warm[:, :], start=True, stop=True
            )

    # first tile that needs each segment -> PE wait right before that
    # tile's first matmul (tile t reads padded rows [8t, 8t+10]). The
    # TileContext scheduler's dry-run sim only covers the in-context block,
    # so these waits would deadlock it if emitted here; record them and
    # inject onto the finished instructions post-hoc (_inject_waits).
    seg_wait_at = {}
    for k, (lo, hi, _) in enumerate(SEGS):
        for t in range(N_TILES):
            if t * TILE_ROWS + TILE_ROWS + 2 > lo:
                seg_wait_at.setdefault(t, []).append(k)
                break

    mm_waits = {}  # matmul emission index -> [sem handles]
    n_mm = N_WARMUP_MM  # warmup matmuls already emitted

    with tile.TileContext(nc) as tc:
        from contextlib import ExitStack

        ctx = ExitStack()
        with ctx:
            psum = ctx.enter_context(tc.tile_pool(name="psum", bufs=2, space="PSUM"))
            ostage = ctx.enter_context(tc.tile_pool(name="ostage", bufs=2))

            groups = (
                [[0]]
                + [[t, t + 1] for t in range(1, N_TILES - 1, G)]
                + [[N_TILES - 1]]
            )
            store_ring = [nc.sync, nc.scalar]
            for gi, tiles in enumerate(groups):
                ps = {
                    t: psum.tile([128, BPC, QROWS, W], f32, name=f"ps{k}")
                    for k, t in enumerate(tiles)
                }
                for n, (i, j) in enumerate(TAPS):
                    for s in range(BPC):
                        for g in range(2):
                            for t in tiles:
                                if n == 0 and s == 0 and g == 0:
                                    for k in seg_wait_at.get(t, ()):
                                        mm_waits.setdefault(n_mm, []).append(s_seg[k])
                                n_mm += 1
                                r0 = t * TILE_ROWS + QROWS * g + i
                                nc.tensor.matmul(
                                    ps[t][64 * g : 64 * g + 64, s, :, :],
                                    lhsT=wts[64 * s : 64 * s + 64, i * KW + j, :],
                                    rhs=raw[
                                        64 * s : 64 * s + 64, r0 : r0 + QROWS, j : j + W
                                    ],
                                    start=(n == 0),
                                    stop=(n == len(TAPS) - 1),
                                )

                # --- evacuate each tile: per sample, the g=s quadrant is
                # partition-aligned with the output slot (DVE +bias), the
                # other crosses partitions (ACT +bias). Both DVE ops before
                # both ACT ops so every cross-engine WAW edge is DVE->ACT.
                for t in tiles:
                    st = ostage.tile([128, TILE_ROWS, W], bf16, name=f"st{t % 4}")
                    last = t == N_TILES - 1
                    r = t * TILE_ROWS
                    for s in range(BPC):
                        home = slice(64 * s, 64 * s + 64)
                        nc.vector.tensor_scalar_add(
                            st[home, QROWS * s : QROWS * s + QROWS, :],
                            ps[t][home, s, :, :],
                            bias_sb[home, :],
                        )
                    for s in (1, 0):
                        home = slice(64 * s, 64 * s + 64)
                        away = slice(64 - 64 * s, 128 - 64 * s)
                        nc.scalar.activation(
                            out=st[home, QROWS * (1 - s) : QROWS * (2 - s), :],
                            in_=ps[t][away, s, :, :],
                            func=mybir.ActivationFunctionType.Identity,
                            bias=bias_sb[home, :],
                        )
                        if last:
                            # st rows 4(1-s):4(2-s) complete: DVE wrote the
                            # home half, this ACT the other half.
                            q0 = QROWS * (1 - s)
                            store_ring[s].dma_start(
                                out=y_bo[:, r + q0 : r + q0 + QROWS, :],
                                in_=st[:, q0 : q0 + QROWS, :],
                            )
                    if not last:
                        store_ring[t % 2].dma_start(
                            out=y_bo[:, r : r + TILE_ROWS, :], in_=st
                        )
    return nc, {"mm": mm_waits, "wts": s_wts, "bias": s_bias}


def _inject_waits(nc, plan):
    """Attach the manual load-completion waits onto the finished program:
    the weights wait onto the first in-context LDWEIGHTS (weights must be
    valid before the load, not the matmul), each row-segment wait onto the
    first matmul of the first tile that reads it, and the bias wait onto
    the first DVE / second ACT op (the first ACTIVATE is the const-input
    table warmer). Runs after _dedup_ldweights (which keeps matmul order)
    and before _split_multi_waits (which hoists multi-wait instructions)."""
    import concourse.mybir as mybir

    def wait_of(sem, value=16):
        return mybir.SyncWait(
            sync_type="semaphore",
            id=sem.num,
            wait_mode="sem-ge-imm",
            ant_name=sem.name,
            wait_value=value,
        )

    def attach(inst, w):
        if inst.sync_info is None:
            inst.sync_info = mybir.SyncInfo(on_wait=[w], on_update=[])
        else:
            inst.sync_info.on_wait.append(w)

    n_mm = n_ldw = n_ts = n_act = 0
    for f in nc.m.functions:
        for blk in f.blocks:
            for inst in blk.instructions:
                if isinstance(inst, mybir.InstMatmult):
                    for sem in plan["mm"].get(n_mm, ()):
                        attach(inst, wait_of(sem))
                    n_mm += 1
                elif isinstance(inst, mybir.InstLdweights):
                    # pre-context warmup matmuls are self-loading (the tile
                    # framework, not bass, splits out InstLdweights), so the
                    # first LDW overall is the first real weight load.
                    if n_ldw == 0:
                        attach(inst, wait_of(plan["wts"]))
                    n_ldw += 1
                elif isinstance(inst, mybir.InstTensorScalarPtr):
                    if n_ts == 0:
                        attach(inst, wait_of(plan["bias"]))
                    n_ts += 1
                elif isinstance(inst, mybir.InstActivation):
                    if n_act == 1:  # 0 = const-input ACT-table warmer
                        attach(inst, wait_of(plan["bias"]))
                    n_act += 1


def _get_module():
    if "nc" not in _CACHE:
        nc, plan = _build_module()
        n = _dedup_ldweights(nc)
        assert n > 0, "expected redundant weight loads to drop"
        _inject_waits(nc, plan)
        _split_multi_waits(nc)
        _CACHE["nc"] = nc
    return _CACHE["nc"]


def _in_maps(input, weight, bias):
    import ml_dtypes

    bf16 = ml_dtypes.bfloat16
    xpad = np.pad(input, ((0, 0), (0, 0), (1, 1), (1, 1)), mode="wrap").astype(bf16)
    maps = []
    for i in range(N_CORES):
        lo, hi = i * BPC, (i + 1) * BPC
        # wts[64b+c, tap, o] = w[b, c*9+tap, o]
        wloc = weight[lo:hi].reshape(BPC, C, KH * KW, O)
        maps.append(
            {
                "input": np.ascontiguousarray(xpad[lo:hi]),
                "wts": np.ascontiguousarray(wloc.reshape(BPC * C, KH * KW, O)).astype(
                    bf16
                ),
                "bias": np.ascontiguousarray(bias[lo:hi]),
            }
        )
    return maps


def kernel(input, weight, bias):
    from concourse.bass_utils import run_bass_kernel_spmd

    nc = _get_module()
    res = run_bass_kernel_spmd(
        nc, _in_maps(input, weight, bias), core_ids=list(range(N_CORES))
    )
    return np.concatenate(
        [res.results[i]["out"] for i in range(N_CORES)], axis=0
    ).astype(np.float32)


# revision 15
# speedup vs baseline: 1.0158x; 1.0158x over previous
"""Trainium2 Bass kernel for per-sample dynamic 3x3 conv (periodic padding).

y[b,o,h,w] = sum_{c,i,j} x[b,c,(h+i-1)%H,(w+j-1)%W] * wgt[b, c*9+i*3+j, o] + bias[b,o]

Shapes: x [16,64,128,128] f32, wgt [16,576,64] f32, bias [16,64] f32.
Sharding: data-parallel over batch, 2 samples per core on 8 cores.

Compute scheme: 64x64 PE-array tiling. Every matmul is K=64 (C), M=64 (O);
the four quadrant matmuls run concurrently on the 128x128 array (measured
216ns per 512-column tap block = 1 col/cycle/quadrant at warm clock).
Quadrant (s,g): sample s on array rows 64s:64s+64, col-group g computes
output rows 4g:4g+4 of an 8-row spatial tile; each quadrant owns a private
(partition x bank) PSUM region so the four accumulation chains start/stop
independently. Tiles are processed in pairs with the tap loop outer within
the pair so _dedup_ldweights drops the second tile's redundant InstLdweights
(bf16 weights stay resident; LDWs background-load under the previous tap's
stream).

The wrap padding is materialized HOST-side ([2,64,130,130] bf16 input), so
every (tap, tile, quadrant) is one clean N=512 matmul - no 1-wide column
slivers (+26ns per affected tap block) and no boundary-tile row splits.

Head: image/weights/bias live in RAW sbuf tensors loaded by DMAs issued
BEFORE the TileContext entry barrier (saves the ~0.8us poison-sem dance),
with manual completion semaphores; the PE stream takes one explicit wait
per row segment, placed right before the first matmul of the first tile
that needs it. The critical pieces (weights + rows 0:11) ride the sync
HWDGE ring, which measured ~0.8us first-touch latency vs ~2us for the
scalar ring. Eight warm-up matmuls on a zeroed scratch (into a PSUM bank
that is freed before the pools allocate; safe because the PE queue is
in-order) keep the PE busy through the DMA head so the HAM clock-gate is
at 2.4GHz when the real stream starts. The ACT table preload runs off a
const input pre-context.

Evacuation per sample: the g=s quadrant is partition-aligned with the
output slot -> DVE tensor_scalar_add(+bias); the g=1-s quadrant crosses
partitions -> ACT activation(Identity, +bias). Tile dep tracking is
partition-blind, so DVE/ACT writes to the same st rows serialize; emitting
both DVE ops before both ACT ops makes every cross-engine edge point
DVE->ACT (never ACT->DVE), keeping the drain to one DVE latency. Stores
alternate between the sync and scalar HWDGE rings (the gpsimd SWDGE ring
measured only ~120GB/s and back-pressured the whole pipeline); the last
tile stores as two 4-row halves on both rings to shorten the final drain.
Inputs/weights are cast to bf16 host-side; output is stored bf16 and
upcast on the host.
"""

import numpy as np

KH = KW = 3
B, C, O, H, W = 16, 64, 64, 128, 128
HP, WP = H + 2, W + 2  # host-padded image
N_CORES = 8
BPC = B // N_CORES  # samples per core
TILE_ROWS = 8  # output rows per spatial tile (4 per quadrant col-group)
QROWS = TILE_ROWS // 2  # rows per quadrant -> N = 4*128 = 512
N_TILES = H // TILE_ROWS
G = 2  # spatial tiles per tap-outer pair (LDW reuse span)
N_WARMUP_MM = 7  # HAM warm-up matmuls (self-loading, ~4.5us cold) in the head

TAPS = [(i, j) for i in range(KH) for j in range(KW)]

# (row_lo, row_hi, ring) row segments of the padded image; tile t needs
# padded rows [8t, 8t+10]. The first ~12us of queue-1 bandwidth measured
# only ~150GB/s, so the head segment is split across both rings.
SEGS = [
    (0, 9, "sync"),
    (9, 11, "scalar"),
    (11, 19, "scalar"),
    (19, 35, "sync"),
    (35, 59, "sync"),
    (59, 91, "sync"),
    (91, HP, "sync"),
]

_CACHE = {}


def _patch_tile_drain():
    """This container's walrus rejects Drain instructions carrying more than
    one sem wait (setupSyncWait: Too many sync wait commands). Re-emit the
    TileContext exit drain's waits as individual wait_ge instructions."""
    import concourse.tile as tile
    from concourse.vector_clock import ScopedClock

    if getattr(tile.TileContext, "_drain_patch_applied", False):
        return

    def _drain_and_barrier(self, tick_clock, wait_clock):
        import concourse.mybir as mybir

        nc = self.nc
        nop = nc.sync.nop(nofuse=True)
        wait_clock.add_sem_waits(nop.ins, ScopedClock({None: tick_clock.global_clock}))
        waits = list(nop.ins.sync_info.on_wait)
        nop.ins.sync_info.on_wait.clear()
        assert self.sems is not None
        by_name = {}
        for h in self.sems.allocated().values():
            by_name[getattr(h, "name", None)] = h
        # Spread the final sem waits round-robin over all engine queues
        # (serial ~115ns/wait on one queue otherwise); the sem-only barrier
        # below restores the all-engines ordering.
        engs = [nc.sync, nc.vector, nc.scalar, nc.tensor, nc.gpsimd]
        for k, w in enumerate(waits):
            h = by_name.get(w.ant_name)
            assert h is not None, f"no sem handle for {w.ant_name}"
            engs[k % len(engs)].wait_ge(h, w.wait_value)
        # per-engine drains except GpSimd's expensive dge_drain, then a
        # sem-only barrier (mirrors BassBlock.no_gpsimd_drain)
        gpsimd_type = nc.gpsimd.engine
        for eng_type, eng in nc.engines.items():
            if eng_type == gpsimd_type:
                continue
            d = mybir.InstDrain(
                name=nc.get_next_instruction_name(),
                ins=[],
                outs=[],
                bass_is_fusable=False,
            )
            d.engine = eng_type
            eng.add_instruction(d)
        nc.all_engine_barrier(sem_only=True)
        popped = nc._tile_sem_poison_stack.pop()
        assert popped is self._sem_poison
        # Skip the exit-time gpsimd dma_reset+sem_clear: the kernel preamble
        # already clears the whole kernel sem range on every execution, and
        # the gpsimd dge drain it implies costs microseconds. Only free the
        # IDs python-side (no further tiles are built after this point).
        sems = list(self.sems.allocated().values())
        sem_nums = [s.num for s in sems]
        nc._state.prepend_free_semaphores(sem_nums)
        for poison_set in nc._tile_sem_poison_stack:
            poison_set.update(sem_nums)

    tile.TileContext._drain_and_barrier = _drain_and_barrier
    tile.TileContext._drain_patch_applied = True


def _split_multi_waits(nc, max_waits=1):
    """Same walrus limitation, general form: any instruction carrying more
    than one sem wait fails setupSyncWait. Hoist excess waits onto dedicated
    single-wait NOPs on the same engine, placed just before the instruction."""
    import concourse.mybir as mybir

    for f in nc.m.functions:
        for blk in f.blocks:
            out = []
            changed = False
            for inst in blk.instructions:
                si = getattr(inst, "sync_info", None)
                waits = list(si.on_wait) if si is not None else []
                if len(waits) > max_waits:
                    changed = True
                    for w in waits[:-max_waits]:
                        out.append(
                            mybir.InstNoOp(
                                name=nc.get_next_instruction_name(),
                                engine=inst.engine,
                                sync_info=mybir.SyncInfo(on_wait=[w], on_update=[]),
                                bass_nofuse=True,
                            )
                        )
                    si.on_wait.clear()
                    for w in waits[-max_waits:]:
                        si.on_wait.append(w)
                out.append(inst)
            if changed:
                blk.instructions = out


def _dedup_ldweights(nc):
    """Drop InstLdweights that reload the exact weights already resident in
    the same PE array quadrant (bf16 weights persist across matmuls; a
    non-self-loading InstMatmult then reuses them - the pattern
    nc.tensor.ldweights documents as supported for 16-bit dtypes). Sync
    carried by a dropped load moves onto the next kept PE instruction."""
    import concourse.mybir as mybir

    def key(inst):
        a = inst.ins[0]
        return (
            a.memref,
            a.offset,
            tuple(tuple(d) for d in a.ap),
            tuple(inst.tile_position or (0, 0)),
            inst.perf_mode,
            inst.is_transpose,
        )

    dropped = 0
    for f in nc.m.functions:
        for blk in f.blocks:
            last = {}
            out = []
            pend_waits = []
            pend_updates = []
            for inst in blk.instructions:
                if getattr(inst, "engine", None) != mybir.EngineType.PE:
                    out.append(inst)
                    continue
                if isinstance(inst, mybir.InstLdweights):
                    pos = tuple(inst.tile_position or (0, 0))
                    k = key(inst)
                    if last.get(pos) == k:
                        si = inst.sync_info
                        if si is not None:
                            pend_waits.extend(si.on_wait)
                            pend_updates.extend(si.on_update)
                        dropped += 1
                        continue
                    last[pos] = k
                elif not isinstance(inst, mybir.InstMatmult):
                    last = {}  # unknown PE inst: conservatively forget
                if pend_waits or pend_updates:
                    if inst.sync_info is None:
                        inst.sync_info = mybir.SyncInfo(on_wait=[], on_update=[])
                    for w in pend_waits:
                        inst.sync_info.on_wait.append(w)
                    for u in pend_updates:
                        inst.sync_info.on_update.append(u)
                    pend_waits, pend_updates = [], []
                out.append(inst)
            assert not pend_waits and not pend_updates
            blk.instructions = out
    return dropped


def _build_module():
    import concourse.bass as bass
    import concourse.mybir as mybir
    import concourse.tile as tile

    _patch_tile_drain()

    f32 = mybir.dt.float32
    bf16 = mybir.dt.bfloat16

    nc = bass.Bass()
    x_d = nc.dram_tensor("input", [BPC, C, HP, WP], bf16, kind="ExternalInput")
    # weights pre-transposed host-side: wts[64*b+c, tap, o]
    w_d = nc.dram_tensor("wts", [128, KH * KW, O], bf16, kind="ExternalInput")
    b_d = nc.dram_tensor("bias", [BPC, O], f32, kind="ExternalInput")
    y_d = nc.dram_tensor("out", [BPC, O, H, W], bf16, kind="ExternalOutput")

    x_bc = x_d.rearrange("b c h w -> (b c) h w")
    y_bo = y_d.rearrange("b o h w -> (b o) h w")

    # --- raw persistent sbuf state, loaded pre-TileContext with manual sems
    raw = nc.alloc_sbuf_tensor("raw", [128, HP, WP], bf16)
    wts = nc.alloc_sbuf_tensor("wtsb", [128, KH * KW, O], bf16)
    bias_sb = nc.alloc_sbuf_tensor("biasb", [128, 1], f32)
    warm = nc.alloc_sbuf_tensor("warm", [128, 512], bf16)
    scratch1 = nc.alloc_sbuf_tensor("scratch1", [128, 1], f32)

    s_wts = nc.alloc_semaphore("s_wts")
    s_bias = nc.alloc_semaphore("s_bias")
    s_warm = nc.alloc_semaphore("s_warm")
    s_seg = [nc.alloc_semaphore(f"s_seg{k}") for k in range(len(SEGS))]

    const0 = nc.const_aps.aps[(f32, 0.0)]

    # zero the warm-up scratch on the otherwise idle DVE (gpsimd's memset +
    # sem propagation measured ~1.7us, delaying the PE warm-up)
    nc.vector.memset(warm[:, :], 0).then_inc(s_warm, 1)

    # critical loads on the sync HWDGE ring (fast first-touch), the rest
    # interleaved; bias + one segment keep the scalar ring's slow first
    # transfer off the critical path.
    nc.sync.dma_start(out=wts[:, :, :], in_=w_d[:, :, :]).then_inc(s_wts, 16)
    ring = {"sync": nc.sync, "scalar": nc.scalar}
    for k, (lo, hi, eng) in enumerate(SEGS):
        ring[eng].dma_start(out=raw[:, lo:hi, :], in_=x_bc[:, lo:hi, :]).then_inc(
            s_seg[k], 16
        )
        if k == 0:
            nc.scalar.dma_start(
                out=bias_sb[:, :],
                in_=b_d.rearrange("b o -> (b o)").rearrange("(p x) -> p x", x=1),
            ).then_inc(s_bias, 16)
            # ACT table preload off a const input (~1.3us, hidden in head)
            nc.scalar.activation(
                out=scratch1[:, :],
                in_=const0,
                func=mybir.ActivationFunctionType.Identity,
                bias=const0,
            )

    # --- PE warm-up on zeros; the PSUM bank is freed before the pools
    # allocate (in-order PE queue makes the overlap safe).
    with nc.psum_tensor([128, 512]) as wps:
        nc.tensor.wait_ge(s_warm, 1)
        for _ in range(N_WARMUP_MM):
            nc.tensor.matmul(
                wps[:, :], lhsT=warm[:, 0:128], rhs=warm[:, :], start=True, stop=True
            )

    # first tile that needs each segment -> PE wait right before that
    # tile's first matmul (tile t reads padded rows [8t, 8t+10]). The
    # TileContext scheduler's dry-run sim only covers the in-context block,
    # so these waits would deadlock it if emitted here; record them and
    # inject onto the finished instructions post-hoc (_inject_waits).
    seg_wait_at = {}
    for k, (lo, hi, _) in enumerate(SEGS):
        for t in range(N_TILES):
            if t * TILE_ROWS + TILE_ROWS + 2 > lo:
                seg_wait_at.setdefault(t, []).append(k)
                break

    mm_waits = {}  # matmul emission index -> [sem handles]
    n_mm = N_WARMUP_MM  # warmup matmuls already emitted

    with tile.TileContext(nc) as tc:
        from contextlib import ExitStack

        ctx = ExitStack()
        with ctx:
            psum = ctx.enter_context(tc.tile_pool(name="psum", bufs=2, space="PSUM"))
            ostage = ctx.enter_context(tc.tile_pool(name="ostage", bufs=2))

            groups = (
                [[0]]
                + [[t, t + 1] for t in range(1, N_TILES - 1, G)]
                + [[N_TILES - 1]]
            )
            for gi, tiles in enumerate(groups):
                ps = {
                    t: psum.tile([128, BPC, QROWS, W], f32, name=f"ps{k}")
                    for k, t in enumerate(tiles)
                }
                for n, (i, j) in enumerate(TAPS):
                    for s in range(BPC):
                        for g in range(2):
                            for t in tiles:
                                if n == 0 and s == 0 and g == 0:
                                    for k in seg_wait_at.get(t, ()):
                                        mm_waits.setdefault(n_mm, []).append(s_seg[k])
                                n_mm += 1
                                r0 = t * TILE_ROWS + QROWS * g + i
                                nc.tensor.matmul(
                                    ps[t][64 * g : 64 * g + 64, s, :, :],
                                    lhsT=wts[64 * s : 64 * s + 64, i * KW + j, :],
                                    rhs=raw[
                                        64 * s : 64 * s + 64, r0 : r0 + QROWS, j : j + W
                                    ],
                                    start=(n == 0),
                                    stop=(n == len(TAPS) - 1),
                                )

                # --- evacuate each tile: per sample, the g=s quadrant is
                # partition-aligned with the output slot (DVE +bias), the
                # other crosses partitions (ACT +bias). Staging uses THREE
                # 4-row slots so DVE (slots 0/2) and ACT (slot 1) write
                # disjoint free ranges - the partition-blind dep tracker
                # then never serializes DVE against ACT, keeping each
                # tile's PSUM-free latency ~1.5us (under the one-group
                # budget that buf rotation imposes). Per partition half the
                # output rows are slots [0,1] (sample 0) / [1,2] (sample
                # 1), stored as two 64-partition DMAs on the sync ring.
                for t in tiles:
                    st = ostage.tile([128, 3, QROWS, W], bf16, name=f"st{t % 4}")
                    r = t * TILE_ROWS
                    for s in range(BPC):
                        home = slice(64 * s, 64 * s + 64)
                        nc.vector.tensor_scalar_add(
                            st[home, 2 * s, :, :],
                            ps[t][home, s, :, :],
                            bias_sb[home, :],
                        )
                    for s in (1, 0):
                        home = slice(64 * s, 64 * s + 64)
                        away = slice(64 - 64 * s, 128 - 64 * s)
                        nc.scalar.activation(
                            out=st[home, 1, :, :],
                            in_=ps[t][away, s, :, :],
                            func=mybir.ActivationFunctionType.Identity,
                            bias=bias_sb[home, :],
                        )
                    nc.sync.dma_start(
                        out=y_bo[0:64, r : r + TILE_ROWS, :],
                        in_=st[0:64, 0:2, :, :],
                    )
                    nc.sync.dma_start(
                        out=y_bo[64:128, r : r + TILE_ROWS, :],
                        in_=st[64:128, 1:3, :, :],
                    )
    return nc, {"mm": mm_waits, "wts": s_wts, "bias": s_bias}


def _inject_waits(nc, plan):
    """Attach the manual load-completion waits onto the finished program:
    the weights wait onto the first in-context LDWEIGHTS (weights must be
    valid before the load, not the matmul), each row-segment wait onto the
    first matmul of the first tile that reads it, and the bias wait onto
    the first DVE / second ACT op (the first ACTIVATE is the const-input
    table warmer). Runs after _dedup_ldweights (which keeps matmul order)
    and before _split_multi_waits (which hoists multi-wait instructions)."""
    import concourse.mybir as mybir

    def wait_of(sem, value=16):
        return mybir.SyncWait(
            sync_type="semaphore",
            id=sem.num,
            wait_mode="sem-ge-imm",
            ant_name=sem.name,
            wait_value=value,
        )

    def attach(inst, w):
        if inst.sync_info is None:
            inst.sync_info = mybir.SyncInfo(on_wait=[w], on_update=[])
        else:
            inst.sync_info.on_wait.append(w)

    n_mm = n_ldw = n_ts = n_act = 0
    for f in nc.m.functions:
        for blk in f.blocks:
            for inst in blk.instructions:
                if isinstance(inst, mybir.InstMatmult):
                    for sem in plan["mm"].get(n_mm, ()):
                        attach(inst, wait_of(sem))
                    n_mm += 1
                elif isinstance(inst, mybir.InstLdweights):
                    # pre-context warmup matmuls are self-loading (the tile
                    # framework, not bass, splits out InstLdweights), so the
                    # first LDW overall is the first real weight load.
                    if n_ldw == 0:
                        attach(inst, wait_of(plan["wts"]))
                    n_ldw += 1
                elif isinstance(inst, mybir.InstTensorScalarPtr):
                    if n_ts == 0:
                        attach(inst, wait_of(plan["bias"]))
                    n_ts += 1
                elif isinstance(inst, mybir.InstActivation):
                    if n_act == 1:  # 0 = const-input ACT-table warmer
                        attach(inst, wait_of(plan["bias"]))
                    n_act += 1


def _get_module():
    if "nc" not in _CACHE:
        nc, plan = _build_module()
        n = _dedup_ldweights(nc)
        assert n > 0, "expected redundant weight loads to drop"
        _inject_waits(nc, plan)
        _split_multi_waits(nc)
        _CACHE["nc"] = nc
    return _CACHE["nc"]


def _in_maps(input, weight, bias):
    import ml_dtypes

    bf16 = ml_dtypes.bfloat16
    xpad = np.pad(input, ((0, 0), (0, 0), (1, 1), (1, 1)), mode="wrap").astype(bf16)
    maps = []
    for i in range(N_CORES):
        lo, hi = i * BPC, (i + 1) * BPC
        # wts[64b+c, tap, o] = w[b, c*9+tap, o]
        wloc = weight[lo:hi].reshape(BPC, C, KH * KW, O)
        maps.append(
            {
                "input": np.ascontiguousarray(xpad[lo:hi]),
                "wts": np.ascontiguousarray(wloc.reshape(BPC * C, KH * KW, O)).astype(
                    bf16
                ),
                "bias": np.ascontiguousarray(bias[lo:hi]),
            }
        )
    return maps


def kernel(input, weight, bias):
    from concourse.bass_utils import run_bass_kernel_spmd

    nc = _get_module()
    res = run_bass_kernel_spmd(
        nc, _in_maps(input, weight, bias), core_ids=list(range(N_CORES))
    )
    return np.concatenate(
        [res.results[i]["out"] for i in range(N_CORES)], axis=0
    ).astype(np.float32)


# revision 25
# speedup vs baseline: 1.1126x; 1.0953x over previous
"""Trainium2 Bass kernel for per-sample dynamic 3x3 conv (periodic padding).

y[b,o,h,w] = sum_{c,i,j} x[b,c,(h+i-1)%H,(w+j-1)%W] * wgt[b, c*9+i*3+j, o] + bias[b,o]

Shapes: x [16,64,128,128] f32, wgt [16,576,64] f32, bias [16,64] f32.
Sharding: data-parallel over batch, 2 samples per core on 8 cores.

Compute scheme: 64x64 PE-array tiling. Every matmul is K=64 (C), M=64 (O);
the four quadrant matmuls run concurrently on the 128x128 array (measured
216ns per 512-column tap block = 1 col/cycle/quadrant at warm clock).
Quadrant (s,g): sample s on array rows 64s:64s+64, col-group g computes
output rows 4g:4g+4 of an 8-row spatial tile; each quadrant owns a private
(partition x bank) PSUM region so the four accumulation chains start/stop
independently. Tiles are processed in pairs with the tap loop outer within
the pair so _dedup_ldweights drops the second tile's redundant InstLdweights
(bf16 weights stay resident; LDWs background-load under the previous tap's
stream).

The wrap padding is materialized HOST-side ([2,64,130,130] bf16 input), so
every (tap, tile, quadrant) is one clean N=512 matmul - no 1-wide column
slivers (+26ns per affected tap block) and no boundary-tile row splits.

Head: image/weights/bias live in RAW sbuf tensors loaded by DMAs issued
BEFORE the TileContext entry barrier (saves the ~0.8us poison-sem dance),
with manual completion semaphores; the PE stream takes one explicit wait
per row segment, placed right before the first matmul of the first tile
that needs it. The critical pieces (weights + rows 0:11) ride the sync
HWDGE ring, which measured ~0.8us first-touch latency vs ~2us for the
scalar ring. Eight warm-up matmuls on a zeroed scratch (into a PSUM bank
that is freed before the pools allocate; safe because the PE queue is
in-order) keep the PE busy through the DMA head so the HAM clock-gate is
at 2.4GHz when the real stream starts. The ACT table preload runs off a
const input pre-context.

Evacuation per sample: the g=s quadrant is partition-aligned with the
output slot -> DVE tensor_scalar_add(+bias); the g=1-s quadrant crosses
partitions -> ACT activation(Identity, +bias). Tile dep tracking is
partition-blind, so DVE/ACT writes to the same st rows serialize; emitting
both DVE ops before both ACT ops makes every cross-engine edge point
DVE->ACT (never ACT->DVE), keeping the drain to one DVE latency. Stores
alternate between the sync and scalar HWDGE rings (the gpsimd SWDGE ring
measured only ~120GB/s and back-pressured the whole pipeline); the last
tile stores as two 4-row halves on both rings to shorten the final drain.
Inputs/weights are cast to bf16 host-side; output is stored bf16 and
upcast on the host.
"""

import numpy as np

KH = KW = 3
B, C, O, H, W = 16, 64, 64, 128, 128
HP, WP = H + 2, W + 2  # host-padded image
N_CORES = 8
BPC = B // N_CORES  # samples per core
TILE_ROWS = 8  # output rows per spatial tile (4 per quadrant col-group)
QROWS = TILE_ROWS // 2  # rows per quadrant -> N = 4*128 = 512
N_TILES = H // TILE_ROWS
G = 2  # spatial tiles per tap-outer pair (LDW reuse span)
N_WARMUP_MM = 10  # HAM warm-up matmuls (self-loading, ~4.4us cold) in the head

TAPS = [(i, j) for i in range(KH) for j in range(KW)]

# (row_lo, row_hi, ring) row segments of the padded image; tile t needs
# padded rows [8t, 8t+10]. The first ~12us of queue-1 bandwidth measured
# only ~150GB/s, so the head segment is split across both rings.
SEGS = [
    (0, 11, "sync"),
    (11, 19, "sync"),
    (19, 35, "sync"),
    (35, 59, "sync"),
    (59, 91, "sync"),
    (91, HP, "sync"),
]

_CACHE = {}


def _patch_tile_drain():
    """This container's walrus rejects Drain instructions carrying more than
    one sem wait (setupSyncWait: Too many sync wait commands). Re-emit the
    TileContext exit drain's waits as individual wait_ge instructions."""
    import concourse.tile as tile
    from concourse.vector_clock import ScopedClock

    if getattr(tile.TileContext, "_drain_patch_applied", False):
        return

    def _drain_and_barrier(self, tick_clock, wait_clock):
        import concourse.mybir as mybir

        nc = self.nc
        nop = nc.sync.nop(nofuse=True)
        wait_clock.add_sem_waits(nop.ins, ScopedClock({None: tick_clock.global_clock}))
        waits = list(nop.ins.sync_info.on_wait)
        nop.ins.sync_info.on_wait.clear()
        assert self.sems is not None
        by_name = {}
        for h in self.sems.allocated().values():
            by_name[getattr(h, "name", None)] = h
        # Spread the final sem waits round-robin over all engine queues
        # (serial ~115ns/wait on one queue otherwise); the sem-only barrier
        # below restores the all-engines ordering.
        engs = [nc.sync, nc.vector, nc.scalar, nc.tensor, nc.gpsimd]
        for k, w in enumerate(waits):
            h = by_name.get(w.ant_name)
            assert h is not None, f"no sem handle for {w.ant_name}"
            engs[k % len(engs)].wait_ge(h, w.wait_value)
        # per-engine drains except GpSimd's expensive dge_drain, then a
        # sem-only barrier (mirrors BassBlock.no_gpsimd_drain)
        gpsimd_type = nc.gpsimd.engine
        for eng_type, eng in nc.engines.items():
            if eng_type == gpsimd_type:
                continue
            d = mybir.InstDrain(
                name=nc.get_next_instruction_name(),
                ins=[],
                outs=[],
                bass_is_fusable=False,
            )
            d.engine = eng_type
            eng.add_instruction(d)
        nc.all_engine_barrier(sem_only=True)
        popped = nc._tile_sem_poison_stack.pop()
        assert popped is self._sem_poison
        # Skip the exit-time gpsimd dma_reset+sem_clear: the kernel preamble
        # already clears the whole kernel sem range on every execution, and
        # the gpsimd dge drain it implies costs microseconds. Only free the
        # IDs python-side (no further tiles are built after this point).
        sems = list(self.sems.allocated().values())
        sem_nums = [s.num for s in sems]
        nc._state.prepend_free_semaphores(sem_nums)
        for poison_set in nc._tile_sem_poison_stack:
            poison_set.update(sem_nums)

    tile.TileContext._drain_and_barrier = _drain_and_barrier
    tile.TileContext._drain_patch_applied = True


def _split_multi_waits(nc, max_waits=1):
    """Same walrus limitation, general form: any instruction carrying more
    than one sem wait fails setupSyncWait. Hoist excess waits onto dedicated
    single-wait NOPs on the same engine, placed just before the instruction."""
    import concourse.mybir as mybir

    for f in nc.m.functions:
        for blk in f.blocks:
            out = []
            changed = False
            for inst in blk.instructions:
                si = getattr(inst, "sync_info", None)
                waits = list(si.on_wait) if si is not None else []
                if len(waits) > max_waits:
                    changed = True
                    for w in waits[:-max_waits]:
                        out.append(
                            mybir.InstNoOp(
                                name=nc.get_next_instruction_name(),
                                engine=inst.engine,
                                sync_info=mybir.SyncInfo(on_wait=[w], on_update=[]),
                                bass_nofuse=True,
                            )
                        )
                    si.on_wait.clear()
                    for w in waits[-max_waits:]:
                        si.on_wait.append(w)
                out.append(inst)
            if changed:
                blk.instructions = out


def _dedup_ldweights(nc):
    """Drop InstLdweights that reload the exact weights already resident in
    the same PE array quadrant (bf16 weights persist across matmuls; a
    non-self-loading InstMatmult then reuses them - the pattern
    nc.tensor.ldweights documents as supported for 16-bit dtypes). Sync
    carried by a dropped load moves onto the next kept PE instruction."""
    import concourse.mybir as mybir

    def key(inst):
        a = inst.ins[0]
        return (
            a.memref,
            a.offset,
            tuple(tuple(d) for d in a.ap),
            tuple(inst.tile_position or (0, 0)),
            inst.perf_mode,
            inst.is_transpose,
        )

    dropped = 0
    for f in nc.m.functions:
        for blk in f.blocks:
            last = {}
            out = []
            pend_waits = []
            pend_updates = []
            for inst in blk.instructions:
                if getattr(inst, "engine", None) != mybir.EngineType.PE:
                    out.append(inst)
                    continue
                if isinstance(inst, mybir.InstLdweights):
                    pos = tuple(inst.tile_position or (0, 0))
                    k = key(inst)
                    if last.get(pos) == k:
                        si = inst.sync_info
                        if si is not None:
                            pend_waits.extend(si.on_wait)
                            pend_updates.extend(si.on_update)
                        dropped += 1
                        continue
                    last[pos] = k
                elif not isinstance(inst, mybir.InstMatmult):
                    last = {}  # unknown PE inst: conservatively forget
                if pend_waits or pend_updates:
                    if inst.sync_info is None:
                        inst.sync_info = mybir.SyncInfo(on_wait=[], on_update=[])
                    for w in pend_waits:
                        inst.sync_info.on_wait.append(w)
                    for u in pend_updates:
                        inst.sync_info.on_update.append(u)
                    pend_waits, pend_updates = [], []
                out.append(inst)
            assert not pend_waits and not pend_updates
            blk.instructions = out
    return dropped


def _build_module():
    import concourse.bass as bass
    import concourse.mybir as mybir
    import concourse.tile as tile

    _patch_tile_drain()

    f32 = mybir.dt.float32
    bf16 = mybir.dt.bfloat16

    nc = bass.Bass()
    x_d = nc.dram_tensor("input", [BPC, C, HP, WP], bf16, kind="ExternalInput")
    # weights pre-transposed host-side: wts[64*b+c, tap, o]
    w_d = nc.dram_tensor("wts", [128, KH * KW, O], bf16, kind="ExternalInput")
    b_d = nc.dram_tensor("bias", [BPC, O], f32, kind="ExternalInput")
    y_d = nc.dram_tensor("out", [BPC, O, H, W], bf16, kind="ExternalOutput")

    x_bc = x_d.rearrange("b c h w -> (b c) h w")
    y_bo = y_d.rearrange("b o h w -> (b o) h w")

    # --- raw persistent sbuf state, loaded pre-TileContext with manual sems
    raw = nc.alloc_sbuf_tensor("raw", [128, HP, WP], bf16)
    wts = nc.alloc_sbuf_tensor("wtsb", [128, KH * KW, O], bf16)
    bias_sb = nc.alloc_sbuf_tensor("biasb", [128, 1], f32)
    scratch1 = nc.alloc_sbuf_tensor("scratch1", [128, 1], f32)

    s_wts = nc.alloc_semaphore("s_wts")
    s_bias = nc.alloc_semaphore("s_bias")
    s_seg = [nc.alloc_semaphore(f"s_seg{k}") for k in range(len(SEGS))]

    const0 = nc.const_aps.aps[(f32, 0.0)]

    # critical loads on the sync HWDGE ring (fast first-touch), the rest
    # interleaved; bias + one segment keep the scalar ring's slow first
    # transfer off the critical path.
    nc.sync.dma_start(out=wts[:, :, :], in_=w_d[:, :, :]).then_inc(s_wts, 16)
    ring = {"sync": nc.sync, "scalar": nc.scalar}
    for k, (lo, hi, eng) in enumerate(SEGS):
        ring[eng].dma_start(out=raw[:, lo:hi, :], in_=x_bc[:, lo:hi, :]).then_inc(
            s_seg[k], 16
        )
        if k == 0:
            nc.scalar.dma_start(
                out=bias_sb[:, :],
                in_=b_d.rearrange("b o -> (b o)").rearrange("(p x) -> p x", x=1),
            ).then_inc(s_bias, 16)
            # ACT table preload off a const input (~1.3us, hidden in head)
            nc.scalar.activation(
                out=scratch1[:, :],
                in_=const0,
                func=mybir.ActivationFunctionType.Identity,
                bias=const0,
            )

    # --- PE warm-up matmuls, issued with NO waits so the PE is busy from
    # the engine preamble on and the HAM clock-gate opens (~3.4us sustained)
    # before the real stream starts. They read the wts SBUF region BEFORE
    # (or while) the weight DMA lands - the values are garbage and may even
    # race the DMA, which is harmless: the results go to a PSUM bank that
    # is never read and freed before the pools allocate (the in-order PE
    # queue orders the overlap), and the real accumulations begin with
    # start=True which overwrites.
    with nc.psum_tensor([128, 512]) as wps:
        for _ in range(N_WARMUP_MM):
            nc.tensor.matmul(
                wps[:, :],
                lhsT=wts[:, 0:2, :],
                rhs=wts[:, 0:8, :],
                start=True,
                stop=True,
            )

    # first tile that needs each segment -> PE wait right before that
    # tile's first matmul (tile t reads padded rows [8t, 8t+10]). The
    # TileContext scheduler's dry-run sim only covers the in-context block,
    # so these waits would deadlock it if emitted here; record them and
    # inject onto the finished instructions post-hoc (_inject_waits).
    seg_wait_at = {}
    for k, (lo, hi, _) in enumerate(SEGS):
        for t in range(N_TILES):
            if t * TILE_ROWS + TILE_ROWS + 2 > lo:
                seg_wait_at.setdefault(t, []).append(k)
                break

    mm_waits = {}  # matmul emission index -> [sem handles]
    n_mm = N_WARMUP_MM  # warmup matmuls already emitted

    with tile.TileContext(nc) as tc:
        from contextlib import ExitStack

        ctx = ExitStack()
        with ctx:
            psum = ctx.enter_context(tc.tile_pool(name="psum", bufs=2, space="PSUM"))
            ostage = ctx.enter_context(tc.tile_pool(name="ostage", bufs=2))

            # last tile runs as two 4-row "mini" passes so the final
            # evac + store drain is half-depth (pure tail latency), and the
            # two tiles before it run as singles so their evacuation (the
            # DVE is ~77% busy per tile) drains under the following tiles'
            # matmul streams instead of piling onto the tail.
            groups = (
                [[0]]
                + [[t, t + 1] for t in range(1, N_TILES - 3, G)]
                + [[N_TILES - 3], [N_TILES - 2]]
            )
            slot_ctr = 0  # rotate PSUM tag names so consecutive single
            # groups never immediately reuse the tag they just released
            for gi, tiles in enumerate(groups):
                # one PSUM tile (= one bank) per (tile, sample): the tile
                # framework serializes all readers of a tile into a chain,
                # so keeping each PSUM tile to exactly 2 readers (one DVE,
                # one ACT) halves the buffer-release latency vs a shared
                # [128, 2, 4, W] tile with 4 chained readers.
                ps = {}
                for t in tiles:
                    for s in range(BPC):
                        ps[(t, s)] = psum.tile(
                            [128, QROWS, W], f32, name=f"ps{slot_ctr % 2}{s}"
                        )
                    slot_ctr += 1
                for n, (i, j) in enumerate(TAPS):
                    for s in range(BPC):
                        for g in range(2):
                            for t in tiles:
                                if n == 0 and s == 0 and g == 0:
                                    for k in seg_wait_at.get(t, ()):
                                        mm_waits.setdefault(n_mm, []).append(s_seg[k])
                                n_mm += 1
                                r0 = t * TILE_ROWS + QROWS * g + i
                                nc.tensor.matmul(
                                    ps[(t, s)][64 * g : 64 * g + 64, :, :],
                                    lhsT=wts[64 * s : 64 * s + 64, i * KW + j, :],
                                    rhs=raw[
                                        64 * s : 64 * s + 64, r0 : r0 + QROWS, j : j + W
                                    ],
                                    start=(n == 0),
                                    stop=(n == len(TAPS) - 1),
                                )

                # --- evacuate each tile: per sample, the g=s quadrant is
                # partition-aligned with the output slot (DVE +bias), the
                # other crosses partitions (ACT +bias). Staging uses THREE
                # 4-row slots so DVE (slots 0/2) and ACT (slot 1) write
                # disjoint free ranges - the partition-blind dep tracker
                # then never serializes DVE against ACT, keeping each
                # tile's PSUM-free latency ~1.5us (under the one-group
                # budget that buf rotation imposes). Per partition half the
                # output rows are slots [0,1] (sample 0) / [1,2] (sample
                # 1), stored as two 64-partition DMAs on the sync ring.
                for t in tiles:
                    st = ostage.tile([128, 3, QROWS, W], bf16, name=f"st{t % 4}")
                    r = t * TILE_ROWS
                    for s in range(BPC):
                        home = slice(64 * s, 64 * s + 64)
                        nc.vector.tensor_scalar_add(
                            st[home, 2 * s, :, :],
                            ps[(t, s)][home, :, :],
                            bias_sb[home, :],
                        )
                    for s in range(BPC):
                        home = slice(64 * s, 64 * s + 64)
                        away = slice(64 - 64 * s, 128 - 64 * s)
                        nc.scalar.activation(
                            out=st[home, 1, :, :],
                            in_=ps[(t, s)][away, :, :],
                            func=mybir.ActivationFunctionType.Identity,
                            bias=bias_sb[home, :],
                        )
                    nc.sync.dma_start(
                        out=y_bo[0:64, r : r + TILE_ROWS, :],
                        in_=st[0:64, 0:2, :, :],
                    )
                    nc.sync.dma_start(
                        out=y_bo[64:128, r : r + TILE_ROWS, :],
                        in_=st[64:128, 1:3, :, :],
                    )

            # --- final tile as two 4-row minis (half-depth tail drain)
            t15 = N_TILES - 1
            for m in range(2):
                rm = t15 * TILE_ROWS + QROWS * m
                psm = {
                    s: psum.tile(
                        [128, QROWS, W], f32, name=f"ps{(slot_ctr + m) % 2}{s}"
                    )
                    for s in range(BPC)
                }
                for n, (i, j) in enumerate(TAPS):
                    for s in range(BPC):
                        for g in range(2):
                            n_mm += 1
                            r0 = rm + 2 * g + i
                            nc.tensor.matmul(
                                psm[s][64 * g : 64 * g + 64, 0:2, :],
                                lhsT=wts[64 * s : 64 * s + 64, i * KW + j, :],
                                rhs=raw[64 * s : 64 * s + 64, r0 : r0 + 2, j : j + W],
                                start=(n == 0),
                                stop=(n == len(TAPS) - 1),
                            )
                st = ostage.tile([128, 3, QROWS, W], bf16, name=f"stm{m}")
                for s in range(BPC):
                    home = slice(64 * s, 64 * s + 64)
                    nc.vector.tensor_scalar_add(
                        st[home, 2 * s, 0:2, :],
                        psm[s][home, 0:2, :],
                        bias_sb[home, :],
                    )
                for s in range(BPC):
                    home = slice(64 * s, 64 * s + 64)
                    away = slice(64 - 64 * s, 128 - 64 * s)
                    nc.scalar.activation(
                        out=st[home, 1, 0:2, :],
                        in_=psm[s][away, 0:2, :],
                        func=mybir.ActivationFunctionType.Identity,
                        bias=bias_sb[home, :],
                    )
                nc.sync.dma_start(
                    out=y_bo[0:64, rm : rm + QROWS, :], in_=st[0:64, 0:2, 0:2, :]
                )
                nc.sync.dma_start(
                    out=y_bo[64:128, rm : rm + QROWS, :], in_=st[64:128, 1:3, 0:2, :]
                )
    return nc, {"mm": mm_waits, "wts": s_wts, "bias": s_bias}


def _inject_waits(nc, plan):
    """Attach the manual load-completion waits onto the finished program:
    the weights wait onto the first in-context LDWEIGHTS (weights must be
    valid before the load, not the matmul), each row-segment wait onto the
    first matmul of the first tile that reads it, and the bias wait onto
    the first DVE / second ACT op (the first ACTIVATE is the const-input
    table warmer). Runs after _dedup_ldweights (which keeps matmul order)
    and before _split_multi_waits (which hoists multi-wait instructions)."""
    import concourse.mybir as mybir

    def wait_of(sem, value=16):
        return mybir.SyncWait(
            sync_type="semaphore",
            id=sem.num,
            wait_mode="sem-ge-imm",
            ant_name=sem.name,
            wait_value=value,
        )

    def attach(inst, w):
        if inst.sync_info is None:
            inst.sync_info = mybir.SyncInfo(on_wait=[w], on_update=[])
        else:
            inst.sync_info.on_wait.append(w)

    n_mm = n_ldw = n_ts = n_act = 0
    for f in nc.m.functions:
        for blk in f.blocks:
            for inst in blk.instructions:
                if isinstance(inst, mybir.InstMatmult):
                    for sem in plan["mm"].get(n_mm, ()):
                        attach(inst, wait_of(sem))
                    n_mm += 1
                elif isinstance(inst, mybir.InstLdweights):
                    # pre-context warmup matmuls are self-loading (the tile
                    # framework, not bass, splits out InstLdweights), so the
                    # first LDW overall is the first real weight load.
                    if n_ldw == 0:
                        attach(inst, wait_of(plan["wts"]))
                    n_ldw += 1
                elif isinstance(inst, mybir.InstTensorScalarPtr):
                    if n_ts == 0:
                        attach(inst, wait_of(plan["bias"]))
                    n_ts += 1
                elif isinstance(inst, mybir.InstActivation):
                    if n_act == 1:  # 0 = const-input ACT-table warmer
                        attach(inst, wait_of(plan["bias"]))
                    n_act += 1


def _get_module():
    if "nc" not in _CACHE:
        nc, plan = _build_module()
        n = _dedup_ldweights(nc)
        assert n > 0, "expected redundant weight loads to drop"
        _inject_waits(nc, plan)
        _split_multi_waits(nc)
        _CACHE["nc"] = nc
    return _CACHE["nc"]


def _in_maps(input, weight, bias):
    import ml_dtypes

    bf16 = ml_dtypes.bfloat16
    xpad = np.pad(input, ((0, 0), (0, 0), (1, 1), (1, 1)), mode="wrap").astype(bf16)
    maps = []
    for i in range(N_CORES):
        lo, hi = i * BPC, (i + 1) * BPC
        # wts[64b+c, tap, o] = w[b, c*9+tap, o]
        wloc = weight[lo:hi].reshape(BPC, C, KH * KW, O)
        maps.append(
            {
                "input": np.ascontiguousarray(xpad[lo:hi]),
                "wts": np.ascontiguousarray(wloc.reshape(BPC * C, KH * KW, O)).astype(
                    bf16
                ),
                "bias": np.ascontiguousarray(bias[lo:hi]),
            }
        )
    return maps


def kernel(input, weight, bias):
    from concourse.bass_utils import run_bass_kernel_spmd

    nc = _get_module()
    res = run_bass_kernel_spmd(
        nc, _in_maps(input, weight, bias), core_ids=list(range(N_CORES))
    )
    return np.concatenate(
        [res.results[i]["out"] for i in range(N_CORES)], axis=0
    ).astype(np.float32)


# revision 28
# speedup vs baseline: 1.1234x; 1.0097x over previous
"""Trainium2 Bass kernel for per-sample dynamic 3x3 conv (periodic padding).

y[b,o,h,w] = sum_{c,i,j} x[b,c,(h+i-1)%H,(w+j-1)%W] * wgt[b, c*9+i*3+j, o] + bias[b,o]

Shapes: x [16,64,128,128] f32, wgt [16,576,64] f32, bias [16,64] f32.
Sharding: data-parallel over batch, 2 samples per core on 8 cores.

Compute scheme: 64x64 PE-array tiling. Every matmul is K=64 (C), M=64 (O);
the four quadrant matmuls run concurrently on the 128x128 array (measured
216ns per 512-column tap block = 1 col/cycle/quadrant at warm clock).
Quadrant (s,g): sample s on array rows 64s:64s+64, col-group g computes
output rows 4g:4g+4 of an 8-row spatial tile; each quadrant owns a private
(partition x bank) PSUM region so the four accumulation chains start/stop
independently. Tiles are processed in pairs with the tap loop outer within
the pair so _dedup_ldweights drops the second tile's redundant InstLdweights
(bf16 weights stay resident; LDWs background-load under the previous tap's
stream).

The wrap padding is materialized HOST-side ([2,64,130,130] bf16 input), so
every (tap, tile, quadrant) is one clean N=512 matmul - no 1-wide column
slivers (+26ns per affected tap block) and no boundary-tile row splits.

Head: image/weights/bias live in RAW sbuf tensors loaded by DMAs issued
BEFORE the TileContext entry barrier (saves the ~0.8us poison-sem dance),
with manual completion semaphores; the PE stream takes one explicit wait
per row segment, placed right before the first matmul of the first tile
that needs it. The critical pieces (weights + rows 0:11) ride the sync
HWDGE ring, which measured ~0.8us first-touch latency vs ~2us for the
scalar ring. Eight warm-up matmuls on a zeroed scratch (into a PSUM bank
that is freed before the pools allocate; safe because the PE queue is
in-order) keep the PE busy through the DMA head so the HAM clock-gate is
at 2.4GHz when the real stream starts. The ACT table preload runs off a
const input pre-context.

Evacuation per sample: the g=s quadrant is partition-aligned with the
output slot -> DVE tensor_scalar_add(+bias); the g=1-s quadrant crosses
partitions -> ACT activation(Identity, +bias). Tile dep tracking is
partition-blind, so DVE/ACT writes to the same st rows serialize; emitting
both DVE ops before both ACT ops makes every cross-engine edge point
DVE->ACT (never ACT->DVE), keeping the drain to one DVE latency. Stores
alternate between the sync and scalar HWDGE rings (the gpsimd SWDGE ring
measured only ~120GB/s and back-pressured the whole pipeline); the last
tile stores as two 4-row halves on both rings to shorten the final drain.
Inputs/weights are cast to bf16 host-side; output is stored bf16 and
upcast on the host.
"""

import numpy as np

KH = KW = 3
B, C, O, H, W = 16, 64, 64, 128, 128
HP, WP = H + 2, W + 2  # host-padded image
N_CORES = 8
BPC = B // N_CORES  # samples per core
TILE_ROWS = 8  # output rows per spatial tile (4 per quadrant col-group)
QROWS = TILE_ROWS // 2  # rows per quadrant -> N = 4*128 = 512
N_TILES = H // TILE_ROWS
G = 2  # spatial tiles per tap-outer pair (LDW reuse span)
N_WARMUP_MM = 10  # HAM warm-up matmuls (self-loading, ~4.4us cold) in the head

TAPS = [(i, j) for i in range(KH) for j in range(KW)]

# Load the image as fp8-e4m3: the PE streams fp8 at bf16 speed (1 col/cyc)
# so compute time is unchanged, but the input DMA bytes halve, pulling the
# head and the early row segments ~2.5us earlier. Weights/bias/output stay
# bf16/f32. MEASURED rel err 2.5e-2 > the 2e-2 gate (and no speedup) -
# kept disabled.
X_FP8 = False

# (row_lo, row_hi, ring) row segments of the padded image; tile t needs
# padded rows [8t, 8t+10]. The first ~12us of queue-1 bandwidth measured
# only ~150GB/s, so the head segment is split across both rings.
SEGS = [
    (0, 11, "sync"),
    (11, 19, "sync"),
    (19, 35, "sync"),
    (35, 59, "sync"),
    (59, 91, "sync"),
    (91, HP, "sync"),
]

_CACHE = {}


def _patch_tile_drain():
    """This container's walrus rejects Drain instructions carrying more than
    one sem wait (setupSyncWait: Too many sync wait commands). Re-emit the
    TileContext exit drain's waits as individual wait_ge instructions."""
    import concourse.tile as tile
    from concourse.vector_clock import ScopedClock

    if getattr(tile.TileContext, "_drain_patch_applied", False):
        return

    def _drain_and_barrier(self, tick_clock, wait_clock):
        import concourse.mybir as mybir

        nc = self.nc
        nop = nc.sync.nop(nofuse=True)
        wait_clock.add_sem_waits(nop.ins, ScopedClock({None: tick_clock.global_clock}))
        waits = list(nop.ins.sync_info.on_wait)
        nop.ins.sync_info.on_wait.clear()
        assert self.sems is not None
        by_name = {}
        for h in self.sems.allocated().values():
            by_name[getattr(h, "name", None)] = h
        # Spread the final sem waits round-robin over all engine queues
        # (serial ~115ns/wait on one queue otherwise); the sem-only barrier
        # below restores the all-engines ordering.
        engs = [nc.sync, nc.vector, nc.scalar, nc.tensor, nc.gpsimd]
        for k, w in enumerate(waits):
            h = by_name.get(w.ant_name)
            assert h is not None, f"no sem handle for {w.ant_name}"
            engs[k % len(engs)].wait_ge(h, w.wait_value)
        # per-engine drains except GpSimd's expensive dge_drain, then a
        # sem-only barrier (mirrors BassBlock.no_gpsimd_drain)
        gpsimd_type = nc.gpsimd.engine
        for eng_type, eng in nc.engines.items():
            if eng_type == gpsimd_type:
                continue
            d = mybir.InstDrain(
                name=nc.get_next_instruction_name(),
                ins=[],
                outs=[],
                bass_is_fusable=False,
            )
            d.engine = eng_type
            eng.add_instruction(d)
        nc.all_engine_barrier(sem_only=True)
        popped = nc._tile_sem_poison_stack.pop()
        assert popped is self._sem_poison
        # Skip the exit-time gpsimd dma_reset+sem_clear: the kernel preamble
        # already clears the whole kernel sem range on every execution, and
        # the gpsimd dge drain it implies costs microseconds. Only free the
        # IDs python-side (no further tiles are built after this point).
        sems = list(self.sems.allocated().values())
        sem_nums = [s.num for s in sems]
        nc._state.prepend_free_semaphores(sem_nums)
        for poison_set in nc._tile_sem_poison_stack:
            poison_set.update(sem_nums)

    tile.TileContext._drain_and_barrier = _drain_and_barrier
    tile.TileContext._drain_patch_applied = True


def _split_multi_waits(nc, max_waits=1):
    """Same walrus limitation, general form: any instruction carrying more
    than one sem wait fails setupSyncWait. Hoist excess waits onto dedicated
    single-wait NOPs on the same engine, placed just before the instruction."""
    import concourse.mybir as mybir

    for f in nc.m.functions:
        for blk in f.blocks:
            out = []
            changed = False
            for inst in blk.instructions:
                si = getattr(inst, "sync_info", None)
                waits = list(si.on_wait) if si is not None else []
                if len(waits) > max_waits:
                    changed = True
                    for w in waits[:-max_waits]:
                        out.append(
                            mybir.InstNoOp(
                                name=nc.get_next_instruction_name(),
                                engine=inst.engine,
                                sync_info=mybir.SyncInfo(on_wait=[w], on_update=[]),
                                bass_nofuse=True,
                            )
                        )
                    si.on_wait.clear()
                    for w in waits[-max_waits:]:
                        si.on_wait.append(w)
                out.append(inst)
            if changed:
                blk.instructions = out


def _dedup_ldweights(nc):
    """Drop InstLdweights that reload the exact weights already resident in
    the same PE array quadrant (bf16 weights persist across matmuls; a
    non-self-loading InstMatmult then reuses them - the pattern
    nc.tensor.ldweights documents as supported for 16-bit dtypes). Sync
    carried by a dropped load moves onto the next kept PE instruction."""
    import concourse.mybir as mybir

    def key(inst):
        a = inst.ins[0]
        return (
            a.memref,
            a.offset,
            tuple(tuple(d) for d in a.ap),
            tuple(inst.tile_position or (0, 0)),
            inst.perf_mode,
            inst.is_transpose,
        )

    dropped = 0
    for f in nc.m.functions:
        for blk in f.blocks:
            last = {}
            out = []
            pend_waits = []
            pend_updates = []
            for inst in blk.instructions:
                if getattr(inst, "engine", None) != mybir.EngineType.PE:
                    out.append(inst)
                    continue
                if isinstance(inst, mybir.InstLdweights):
                    pos = tuple(inst.tile_position or (0, 0))
                    k = key(inst)
                    if last.get(pos) == k:
                        si = inst.sync_info
                        if si is not None:
                            pend_waits.extend(si.on_wait)
                            pend_updates.extend(si.on_update)
                        dropped += 1
                        continue
                    last[pos] = k
                elif not isinstance(inst, mybir.InstMatmult):
                    last = {}  # unknown PE inst: conservatively forget
                if pend_waits or pend_updates:
                    if inst.sync_info is None:
                        inst.sync_info = mybir.SyncInfo(on_wait=[], on_update=[])
                    for w in pend_waits:
                        inst.sync_info.on_wait.append(w)
                    for u in pend_updates:
                        inst.sync_info.on_update.append(u)
                    pend_waits, pend_updates = [], []
                out.append(inst)
            assert not pend_waits and not pend_updates
            blk.instructions = out
    return dropped


def _build_module():
    import concourse.bass as bass
    import concourse.mybir as mybir
    import concourse.tile as tile

    _patch_tile_drain()

    f32 = mybir.dt.float32
    bf16 = mybir.dt.bfloat16

    fp8 = mybir.dt.float8e4
    x_dt = fp8 if X_FP8 else bf16

    nc = bass.Bass()
    x_d = nc.dram_tensor("input", [BPC, C, HP, WP], x_dt, kind="ExternalInput")
    # weights pre-transposed host-side: wts[64*b+c, tap, o]
    w_d = nc.dram_tensor("wts", [128, KH * KW, O], bf16, kind="ExternalInput")
    b_d = nc.dram_tensor("bias", [BPC, O], f32, kind="ExternalInput")
    y_d = nc.dram_tensor("out", [BPC, O, H, W], bf16, kind="ExternalOutput")

    x_bc = x_d.rearrange("b c h w -> (b c) h w")
    y_bo = y_d.rearrange("b o h w -> (b o) h w")

    # --- raw persistent sbuf state, loaded pre-TileContext with manual sems
    raw = nc.alloc_sbuf_tensor("raw", [128, HP, WP], x_dt)
    wts = nc.alloc_sbuf_tensor("wtsb", [128, KH * KW, O], bf16)
    bias_sb = nc.alloc_sbuf_tensor("biasb", [128, 1], f32)
    scratch1 = nc.alloc_sbuf_tensor("scratch1", [128, 1], f32)

    s_wts = nc.alloc_semaphore("s_wts")
    s_bias = nc.alloc_semaphore("s_bias")
    s_seg = [nc.alloc_semaphore(f"s_seg{k}") for k in range(len(SEGS))]

    const0 = nc.const_aps.aps[(f32, 0.0)]

    # critical loads on the sync HWDGE ring (fast first-touch), the rest
    # interleaved; bias + one segment keep the scalar ring's slow first
    # transfer off the critical path.
    nc.sync.dma_start(out=wts[:, :, :], in_=w_d[:, :, :]).then_inc(s_wts, 16)
    ring = {"sync": nc.sync, "scalar": nc.scalar}
    for k, (lo, hi, eng) in enumerate(SEGS):
        ring[eng].dma_start(out=raw[:, lo:hi, :], in_=x_bc[:, lo:hi, :]).then_inc(
            s_seg[k], 16
        )
        if k == 0:
            nc.scalar.dma_start(
                out=bias_sb[:, :],
                in_=b_d.rearrange("b o -> (b o)").rearrange("(p x) -> p x", x=1),
            ).then_inc(s_bias, 16)
            # ACT table preload off a const input (~1.3us, hidden in head)
            nc.scalar.activation(
                out=scratch1[:, :],
                in_=const0,
                func=mybir.ActivationFunctionType.Identity,
                bias=const0,
            )

    # --- PE warm-up matmuls, issued with NO waits so the PE is busy from
    # the engine preamble on and the HAM clock-gate opens (~3.4us sustained)
    # before the real stream starts. They read the wts SBUF region BEFORE
    # (or while) the weight DMA lands - the values are garbage and may even
    # race the DMA, which is harmless: the results go to a PSUM bank that
    # is never read and freed before the pools allocate (the in-order PE
    # queue orders the overlap), and the real accumulations begin with
    # start=True which overwrites.
    with nc.psum_tensor([128, 512]) as wps:
        for _ in range(N_WARMUP_MM):
            nc.tensor.matmul(
                wps[:, :],
                lhsT=wts[:, 0:2, :],
                rhs=wts[:, 0:8, :],
                start=True,
                stop=True,
            )

    # first tile that needs each segment -> PE wait right before that
    # tile's first matmul (tile t reads padded rows [8t, 8t+10]). The
    # TileContext scheduler's dry-run sim only covers the in-context block,
    # so these waits would deadlock it if emitted here; record them and
    # inject onto the finished instructions post-hoc (_inject_waits).
    seg_wait_at = {}
    for k, (lo, hi, _) in enumerate(SEGS):
        for t in range(N_TILES):
            if t * TILE_ROWS + TILE_ROWS + 2 > lo:
                seg_wait_at.setdefault(t, []).append(k)
                break

    mm_waits = {}  # matmul emission index -> [sem handles]
    n_mm = N_WARMUP_MM  # warmup matmuls already emitted

    with tile.TileContext(nc) as tc:
        from contextlib import ExitStack

        ctx = ExitStack()
        with ctx:
            psum = ctx.enter_context(tc.tile_pool(name="psum", bufs=2, space="PSUM"))
            ostage = ctx.enter_context(tc.tile_pool(name="ostage", bufs=2))

            # last tile runs as two 4-row "mini" passes so the final
            # evac + store drain is half-depth (pure tail latency), and the
            # two tiles before it run as singles so their evacuation (the
            # DVE is ~77% busy per tile) drains under the following tiles'
            # matmul streams instead of piling onto the tail.
            groups = (
                [[0]]
                + [[t, t + 1] for t in range(1, N_TILES - 5, G)]
                + [[t] for t in range(N_TILES - 5, N_TILES - 1)]
            )
            slot_ctr = 0  # rotate PSUM tag names so consecutive single
            # groups never immediately reuse the tag they just released
            for gi, tiles in enumerate(groups):
                # one PSUM tile (= one bank) per (tile, sample): the tile
                # framework serializes all readers of a tile into a chain,
                # so keeping each PSUM tile to exactly 2 readers (one DVE,
                # one ACT) halves the buffer-release latency vs a shared
                # [128, 2, 4, W] tile with 4 chained readers.
                ps = {}
                for t in tiles:
                    for s in range(BPC):
                        ps[(t, s)] = psum.tile(
                            [128, QROWS, W], f32, name=f"ps{slot_ctr % 2}{s}"
                        )
                    slot_ctr += 1
                for n, (i, j) in enumerate(TAPS):
                    for s in range(BPC):
                        for g in range(2):
                            for t in tiles:
                                if n == 0 and s == 0 and g == 0:
                                    for k in seg_wait_at.get(t, ()):
                                        mm_waits.setdefault(n_mm, []).append(s_seg[k])
                                n_mm += 1
                                r0 = t * TILE_ROWS + QROWS * g + i
                                nc.tensor.matmul(
                                    ps[(t, s)][64 * g : 64 * g + 64, :, :],
                                    lhsT=wts[64 * s : 64 * s + 64, i * KW + j, :],
                                    rhs=raw[
                                        64 * s : 64 * s + 64, r0 : r0 + QROWS, j : j + W
                                    ],
                                    start=(n == 0),
                                    stop=(n == len(TAPS) - 1),
                                )

                # --- evacuate each tile: per sample, the g=s quadrant is
                # partition-aligned with the output slot (DVE +bias), the
                # other crosses partitions (ACT +bias). Staging uses THREE
                # 4-row slots so DVE (slots 0/2) and ACT (slot 1) write
                # disjoint free ranges - the partition-blind dep tracker
                # then never serializes DVE against ACT, keeping each
                # tile's PSUM-free latency ~1.5us (under the one-group
                # budget that buf rotation imposes). Per partition half the
                # output rows are slots [0,1] (sample 0) / [1,2] (sample
                # 1), stored as two 64-partition DMAs on the sync ring.
                for t in tiles:
                    st = ostage.tile([128, 3, QROWS, W], bf16, name=f"st{t % 4}")
                    r = t * TILE_ROWS
                    for s in range(BPC):
                        home = slice(64 * s, 64 * s + 64)
                        nc.vector.tensor_scalar_add(
                            st[home, 2 * s, :, :],
                            ps[(t, s)][home, :, :],
                            bias_sb[home, :],
                        )
                    for s in range(BPC):
                        home = slice(64 * s, 64 * s + 64)
                        away = slice(64 - 64 * s, 128 - 64 * s)
                        nc.scalar.activation(
                            out=st[home, 1, :, :],
                            in_=ps[(t, s)][away, :, :],
                            func=mybir.ActivationFunctionType.Identity,
                            bias=bias_sb[home, :],
                        )
                    nc.sync.dma_start(
                        out=y_bo[0:64, r : r + TILE_ROWS, :],
                        in_=st[0:64, 0:2, :, :],
                    )
                    nc.sync.dma_start(
                        out=y_bo[64:128, r : r + TILE_ROWS, :],
                        in_=st[64:128, 1:3, :, :],
                    )

            # --- final tile as two 4-row minis (half-depth tail drain)
            t15 = N_TILES - 1
            for m in range(2):
                rm = t15 * TILE_ROWS + QROWS * m
                psm = {
                    s: psum.tile(
                        [128, QROWS, W], f32, name=f"ps{(slot_ctr + m) % 2}{s}"
                    )
                    for s in range(BPC)
                }
                for n, (i, j) in enumerate(TAPS):
                    for s in range(BPC):
                        for g in range(2):
                            n_mm += 1
                            r0 = rm + 2 * g + i
                            nc.tensor.matmul(
                                psm[s][64 * g : 64 * g + 64, 0:2, :],
                                lhsT=wts[64 * s : 64 * s + 64, i * KW + j, :],
                                rhs=raw[64 * s : 64 * s + 64, r0 : r0 + 2, j : j + W],
                                start=(n == 0),
                                stop=(n == len(TAPS) - 1),
                            )
                st = ostage.tile([128, 3, QROWS, W], bf16, name=f"stm{m}")
                for s in range(BPC):
                    home = slice(64 * s, 64 * s + 64)
                    nc.vector.tensor_scalar_add(
                        st[home, 2 * s, 0:2, :],
                        psm[s][home, 0:2, :],
                        bias_sb[home, :],
                    )
                for s in range(BPC):
                    home = slice(64 * s, 64 * s + 64)
                    away = slice(64 - 64 * s, 128 - 64 * s)
                    nc.scalar.activation(
                        out=st[home, 1, 0:2, :],
                        in_=psm[s][away, 0:2, :],
                        func=mybir.ActivationFunctionType.Identity,
                        bias=bias_sb[home, :],
                    )
                nc.sync.dma_start(
                    out=y_bo[0:64, rm : rm + QROWS, :], in_=st[0:64, 0:2, 0:2, :]
                )
                nc.sync.dma_start(
                    out=y_bo[64:128, rm : rm + QROWS, :], in_=st[64:128, 1:3, 0:2, :]
                )
    return nc, {"mm": mm_waits, "wts": s_wts, "bias": s_bias}


def _inject_waits(nc, plan):
    """Attach the manual load-completion waits onto the finished program:
    the weights wait onto the first in-context LDWEIGHTS (weights must be
    valid before the load, not the matmul), each row-segment wait onto the
    first matmul of the first tile that reads it, and the bias wait onto
    the first DVE / second ACT op (the first ACTIVATE is the const-input
    table warmer). Runs after _dedup_ldweights (which keeps matmul order)
    and before _split_multi_waits (which hoists multi-wait instructions)."""
    import concourse.mybir as mybir

    def wait_of(sem, value=16):
        return mybir.SyncWait(
            sync_type="semaphore",
            id=sem.num,
            wait_mode="sem-ge-imm",
            ant_name=sem.name,
            wait_value=value,
        )

    def attach(inst, w):
        if inst.sync_info is None:
            inst.sync_info = mybir.SyncInfo(on_wait=[w], on_update=[])
        else:
            inst.sync_info.on_wait.append(w)

    n_mm = n_ldw = n_ts = n_act = 0
    for f in nc.m.functions:
        for blk in f.blocks:
            for inst in blk.instructions:
                if isinstance(inst, mybir.InstMatmult):
                    for sem in plan["mm"].get(n_mm, ()):
                        attach(inst, wait_of(sem))
                    n_mm += 1
                elif isinstance(inst, mybir.InstLdweights):
                    # pre-context warmup matmuls are self-loading (the tile
                    # framework, not bass, splits out InstLdweights), so the
                    # first LDW overall is the first real weight load.
                    if n_ldw == 0:
                        attach(inst, wait_of(plan["wts"]))
                    n_ldw += 1
                elif isinstance(inst, mybir.InstTensorScalarPtr):
                    if n_ts == 0:
                        attach(inst, wait_of(plan["bias"]))
                    n_ts += 1
                elif isinstance(inst, mybir.InstActivation):
                    if n_act == 1:  # 0 = const-input ACT-table warmer
                        attach(inst, wait_of(plan["bias"]))
                    n_act += 1


def _get_module():
    if "nc" not in _CACHE:
        nc, plan = _build_module()
        n = _dedup_ldweights(nc)
        assert n > 0, "expected redundant weight loads to drop"
        _inject_waits(nc, plan)
        _split_multi_waits(nc)
        _CACHE["nc"] = nc
    return _CACHE["nc"]


def _in_maps(input, weight, bias):
    import ml_dtypes

    import concourse.mybir as mybir

    bf16 = ml_dtypes.bfloat16
    x_np = mybir.dt.np(mybir.dt.float8e4) if X_FP8 else bf16
    xpad = np.pad(input, ((0, 0), (0, 0), (1, 1), (1, 1)), mode="wrap").astype(x_np)
    maps = []
    for i in range(N_CORES):
        lo, hi = i * BPC, (i + 1) * BPC
        # wts[64b+c, tap, o] = w[b, c*9+tap, o]
        wloc = weight[lo:hi].reshape(BPC, C, KH * KW, O)
        maps.append(
            {
                "input": np.ascontiguousarray(xpad[lo:hi]),
                "wts": np.ascontiguousarray(wloc.reshape(BPC * C, KH * KW, O)).astype(
                    bf16
                ),
                "bias": np.ascontiguousarray(bias[lo:hi]),
            }
        )
    return maps


def kernel(input, weight, bias):
    from concourse.bass_utils import run_bass_kernel_spmd

    nc = _get_module()
    res = run_bass_kernel_spmd(
        nc, _in_maps(input, weight, bias), core_ids=list(range(N_CORES))
    )
    return np.concatenate(
        [res.results[i]["out"] for i in range(N_CORES)], axis=0
    ).astype(np.float32)
